# revision 15
# baseline (speedup 1.0000x reference)
"""Trainium2 Bass kernel for nn_CADense (context-adaptive low-rank dense layer).

Computes, for the full batch:
    s_mod = s + context @ w          # [B, R]
    low   = (data @ u) * s_mod       # [B, R]
    out   = relu(low @ v.T + 2*bias) # [B, UNITS]

Sharding: data-parallel over batch across 8 NeuronCores; u/s/v/w/bias
replicated. Each core runs the same Bass program on its 1024-row shard.

The kernel sits at the bf16 ridge: ~31us of PE streaming (73728 512-col
matmul columns at 2.4GHz) vs ~33us of HBM traffic (7.25 MiB loads +
4 MiB stores at ~350-430 GB/s/core). Schedule:

- Loads are chopped into ~0.25 MiB quanta and strictly alternated
  between the sync and scalar HWDGE rings IN CONSUMPTION ORDER: each
  ring's FIFO order is the consumption order (ring order = priority),
  both rings carry equal byte prefixes at every point (no priority
  inversion), push latency halves, and the two queues together can
  exceed the single-queue ~353 GB/s.
- Phase 1 (DMA-bound): smod0 + rank0 stream data as it arrives; the
  ~3us 1.2GHz HAM ramp hides inside the DMA pacing. Garbage warm-up
  matmuls bridge the initial descriptor-push latency, and garbage
  "keeper" matmuls plug the remaining bubbles - any PE idle gap over
  ~2us makes the HAM power manager throttle the PE to 50% for
  microseconds at a time.
- Phase 2 (PE-bound): out halves (relu(vT.T @ lowT + 2b) per (m-chunk,
  batch-half)) interleave with the DMA-paced rank1 stream; after mul1
  the remaining halves complete each m-chunk's full-batch row block,
  which stores with 2KB-contiguous runs (2x the descriptor efficiency
  of half-batch stores), FIFO behind the loads on the two rings; the
  last chunks ride the idle gpsimd ring.
- outT stays transposed so the 2*bias term is a per-partition scalar
  fused into PSUM evacuation (scalar.activation bias / DVE max+add),
  alternating scalar/vector per half so neither engine serializes.
"""

import os
import sys
from contextlib import ExitStack

import numpy as np
import ml_dtypes


def _ensure_concourse():
    try:
        import concourse  # noqa: F401
    except ImportError:
        for p in ("/opt/trn_rl_repo", "/root/.axon_site/_ro/trn_rl_repo"):
            if os.path.isdir(p) and p not in sys.path:
                sys.path.insert(0, p)


_ensure_concourse()

import concourse.tile as tile  # noqa: E402
from concourse import bacc, mybir  # noqa: E402
from concourse.bass_utils import run_bass_kernel_spmd  # noqa: E402

NCORES = 8
B, N_IN, UNITS, RANK, CCTX = 8192, 2048, 2048, 256, 512
NB = B // NCORES  # batch rows per core
P = 128
BT = 512  # batch tile (free dim of rank matmuls, PSUM bank width)
NBT = NB // BT  # 2 batch tiles per core
KC = N_IN // P  # 16 contraction chunks for data @ u
CC = CCTX // P  # 4 contraction chunks for context @ w
RC = RANK // P  # 2 rank chunks
MC = UNITS // P  # 16 output unit chunks (partition dim of outT)
N_WARMUP_MM = 9  # 512-col garbage matmuls bridging the push-latency window

F32 = mybir.dt.float32
BF16 = mybir.dt.bfloat16
NP_BF16 = np.dtype(ml_dtypes.bfloat16)


def _emit(nc, tc, ctx):
    d_dataT = nc.dram_tensor("dataT", [N_IN, NB], BF16, kind="ExternalInput")
    d_ctxT = nc.dram_tensor("ctxT", [CCTX, NB], BF16, kind="ExternalInput")
    d_u = nc.dram_tensor("u", [N_IN, RANK], BF16, kind="ExternalInput")
    d_s = nc.dram_tensor("s", [RANK], F32, kind="ExternalInput")
    d_vT = nc.dram_tensor("vT", [RANK, UNITS], BF16, kind="ExternalInput")
    d_w = nc.dram_tensor("w", [CCTX, RANK], BF16, kind="ExternalInput")
    d_bias = nc.dram_tensor("bias", [UNITS], F32, kind="ExternalInput")
    d_outT = nc.dram_tensor("outT", [UNITS, NB], BF16, kind="ExternalOutput")

    ap_dataT = d_dataT.ap().rearrange("(c p) b -> p c b", p=P)
    ap_ctxT = d_ctxT.ap().rearrange("(cc p) b -> p cc b", p=P)
    ap_u = d_u.ap().rearrange("(q p) r -> p q r", p=P)
    ap_w = d_w.ap().rearrange("(cc p) r -> p cc r", p=P)
    ap_vT = d_vT.ap().rearrange("(rc p) m -> p rc m", p=P)
    ap_outT = d_outT.ap().rearrange("(mc p) b -> p mc b", p=P)

    singles = ctx.enter_context(tc.tile_pool(name="singles", bufs=1))
    du_psum = ctx.enter_context(tc.tile_pool(name="du_psum", bufs=2, space="PSUM"))
    s_psum = ctx.enter_context(tc.tile_pool(name="s_psum", bufs=2, space="PSUM"))
    o_psum = ctx.enter_context(tc.tile_pool(name="o_psum", bufs=4, space="PSUM"))

    # Warm-up operands, memset on the (otherwise idle) gpsimd engine so
    # the first LDWEIGHTS can issue right after the entry barrier.
    wu_a = singles.tile([P, P], BF16, name="wu_a")
    wu_b = singles.tile([P, BT], BF16, name="wu_b")
    nc.gpsimd.memset(wu_a[:], 1.0)
    nc.gpsimd.memset(wu_b[:], 1.0)

    # ---- SBUF tiles ----------------------------------------------------
    u_t = {g: singles.tile([P, 4, RANK], BF16, name=f"u{g}") for g in range(4)}
    # All data in 2-kc chunks (0.25 MiB quanta for the 2-ring alternation).
    d_t = {
        (bt, g): singles.tile([P, 2, BT], BF16, name=f"d{bt}g{g}")
        for bt in range(NBT)
        for g in range(KC // 2)
    }

    def data_chunk(bt, kc):
        return d_t[(bt, kc // 2)][:, kc % 2, :]

    s_sb = singles.tile([P, RC], F32, name="s_sb")
    bias_sb = singles.tile([P, MC], F32, name="bias_sb")
    bias2 = singles.tile([P, MC], F32, name="bias2")
    nbias2 = singles.tile([P, MC], F32, name="nbias2")
    w_sb = singles.tile([P, CC, RANK], BF16, name="w_sb")
    ctx_t = {bt: singles.tile([P, CC, BT], BF16, name=f"ctx{bt}") for bt in range(NBT)}
    vT_sb = singles.tile([P, RC, UNITS], BF16, name="vT_sb")

    smod = singles.tile([P, RC, NB], F32, name="smod")
    lowT = {
        (bt, rc): singles.tile([P, BT], BF16, name=f"lowT{bt}r{rc}")
        for bt in range(NBT)
        for rc in range(RC)
    }
    # Full-batch output staging: one row block per m-chunk, so the store
    # for m-chunk mc is a single [P, NB] transfer with 2KB-contiguous
    # destination rows.
    osb = {mc: singles.tile([P, NB], BF16, name=f"osb{mc}") for mc in range(MC)}

    # ---- load queues: ~0.25 MiB quanta alternating sync/gpsimd ---------
    # NOT scalar: scalar is the evacuation engine, and load pushes (which
    # block on ring-full/sem-recycle waits) would serialize ahead of the
    # evacuations in its in-order queue, stalling the whole pipeline.
    _ring = [0]

    def ld(out, in_):
        eng = nc.sync if _ring[0] % 2 == 0 else nc.gpsimd
        _ring[0] += 1
        eng.dma_start(out=out, in_=in_)

    def load_data(bt, g):
        ld(d_t[(bt, g)][:], ap_dataT[:, 2 * g : 2 * g + 2, bt * BT : (bt + 1) * BT])

    ld(ctx_t[0][:, 0:2, :], ap_ctxT[:, 0:2, 0:BT])
    ld(w_sb[:], ap_w)
    ld(u_t[0][:], ap_u[:, 0:4])
    load_data(0, 0)  # kc0-1
    ld(ctx_t[0][:, 2:4, :], ap_ctxT[:, 2:4, 0:BT])
    load_data(0, 1)  # kc2-3
    ld(s_sb[:], d_s.ap().rearrange("(rc p) -> p rc", p=P))
    ld(u_t[1][:], ap_u[:, 4:8])
    load_data(0, 2)  # kc4-5
    ld(bias_sb[:], d_bias.ap().rearrange("(mc p) -> p mc", p=P))
    load_data(0, 3)  # kc6-7
    ld(u_t[2][:], ap_u[:, 8:12])
    load_data(0, 4)  # kc8-9
    load_data(0, 5)  # kc10-11
    ld(u_t[3][:], ap_u[:, 12:16])
    load_data(0, 6)  # kc12-13
    load_data(0, 7)  # kc14-15
    ld(ctx_t[1][:, 0:2, :], ap_ctxT[:, 0:2, BT:])
    ld(ctx_t[1][:, 2:4, :], ap_ctxT[:, 2:4, BT:])
    ld(vT_sb[:, :, 0:1024], ap_vT[:, :, 0:1024])  # mc0-7, both rc
    ld(vT_sb[:, :, 1024:2048], ap_vT[:, :, 1024:2048])  # mc8-15
    for g in range(8):
        load_data(1, g)

    # ---- compute stages ------------------------------------------------
    wu_ps = o_psum.tile([P, BT], F32, tag="po", name="wu_ps")

    def emit_warmups(n):
        for _ in range(n):
            nc.tensor.matmul(wu_ps[:], lhsT=wu_a[:], rhs=wu_b[:], start=True, stop=True)

    def emit_keepers(n):
        """Small garbage matmuls that keep the PE busy across DMA-paced
        bubbles so the HAM power manager never sees a long idle gap."""
        for _ in range(n):
            nc.tensor.matmul(
                wu_ps[:, 0:P], lhsT=wu_a[:], rhs=wu_b[:, 0:P], start=True, stop=True
            )

    pd_t = {}
    ps_t = {}

    def emit_smod_mms(bt, cc_lo, cc_hi):
        """ctx @ w matmuls, cc-outer so they chase the ctx halves."""
        if cc_lo == 0:
            ps_t[bt] = [s_psum.tile([P, BT], F32, tag="ps", name="ps") for _ in range(RC)]
        for cc in range(cc_lo, cc_hi):
            for rc in range(RC):
                nc.tensor.matmul(
                    ps_t[bt][rc][:],
                    lhsT=w_sb[:, cc, rc * P : (rc + 1) * P],
                    rhs=ctx_t[bt][:, cc, :],
                    start=(cc == 0),
                    stop=(cc == CC - 1),
                )

    def emit_smod_evac(bt):
        for rc in range(RC):
            nc.scalar.add(
                smod[:, rc, bt * BT : (bt + 1) * BT],
                ps_t[bt][rc][:],
                add=s_sb[:, rc : rc + 1],
            )

    def emit_rank_mms(bt, kc_lo, kc_hi, keepers=0):
        if kc_lo == 0:
            pd_t[bt] = [du_psum.tile([P, BT], F32, tag="pd", name="pd") for _ in range(RC)]
        for kc in range(kc_lo, kc_hi):
            for rc in range(RC):
                nc.tensor.matmul(
                    pd_t[bt][rc][:],
                    lhsT=u_t[kc // 4][:, kc % 4, rc * P : (rc + 1) * P],
                    rhs=data_chunk(bt, kc),
                    start=(kc == 0),
                    stop=(kc == KC - 1),
                )
            if keepers and kc % 2 == 1:
                emit_keepers(keepers)

    def emit_mul(bt):
        """lowT = pd * smod on the vector engine (bf16 out)."""
        for rc in range(RC):
            nc.vector.tensor_mul(
                out=lowT[(bt, rc)][:],
                in0=pd_t[bt][rc][:],
                in1=smod[:, rc, bt * BT : (bt + 1) * BT],
            )

    # Output halves: outT[mc, bt-half] = relu(vT.T @ lowT + 2*bias) into a
    # rotating PSUM bank, evacuated into the full-batch osb[mc] row block.
    _opools = [(o_psum, "po"), (s_psum, "ps"), (o_psum, "po"), (du_psum, "pd")]
    _oidx = [0]

    def emit_out_half(mc, bt, borrow_all):
        if borrow_all:
            pool, tag = _opools[_oidx[0] % 4]
        else:
            pool, tag = o_psum, "po"
        _oidx[0] += 1
        po = pool.tile([P, BT], F32, tag=tag, name="po")
        for rc in range(RC):
            nc.tensor.matmul(
                po[:],
                lhsT=vT_sb[:, rc, mc * P : (mc + 1) * P],
                rhs=lowT[(bt, rc)][:],
                start=(rc == 0),
                stop=(rc == RC - 1),
            )
        dst = osb[mc][:, bt * BT : (bt + 1) * BT]
        if (mc + bt) % 2 == 0:
            nc.scalar.activation(
                dst,
                po[:],
                mybir.ActivationFunctionType.Relu,
                bias=bias2[:, mc : mc + 1],
            )
        else:
            nc.vector.tensor_scalar(
                out=dst,
                in0=po[:],
                scalar1=nbias2[:, mc : mc + 1],
                scalar2=bias2[:, mc : mc + 1],
                op0=mybir.AluOpType.max,
                op1=mybir.AluOpType.add,
            )

    def store_mc(mc, eng=None):
        eng = eng or nc.sync
        eng.dma_start(out=ap_outT[:, mc, :], in_=osb[mc][:])

    # ---- software pipeline, PE emission in DMA-arrival order -----------
    emit_warmups(N_WARMUP_MM)
    emit_smod_mms(0, 0, 2)  # ctx0 first half + w
    emit_rank_mms(0, 0, 2, keepers=1)  # u0, d0 kc0-1
    emit_smod_mms(0, 2, 4)  # ctx0 second half
    emit_smod_evac(0)
    nc.scalar.mul(bias2[:], bias_sb[:], 2.0)
    nc.scalar.mul(nbias2[:], bias_sb[:], -2.0)
    emit_rank_mms(0, 2, 16, keepers=1)  # d0 kc2-15
    emit_mul(0)
    emit_keepers(8)
    emit_smod_mms(1, 0, 4)  # ctx1
    emit_smod_evac(1)
    emit_keepers(8)
    # Phase 2a: bt0 output halves (need only mul0 + vT) interleaved with
    # the DMA-paced rank1 stream.
    emit_out_half(0, 0, False)
    emit_out_half(1, 0, False)
    for g in range(8):
        emit_rank_mms(1, 2 * g, 2 * g + 2)
        if g < 8:
            emit_out_half(2 + g, 0, False)
    emit_mul(1)
    # Phase 2b: remaining bt0 halves, then bt1 halves completing each
    # m-chunk row block, stored immediately with 2KB-contiguous runs.
    for mc in range(10, MC):
        emit_out_half(mc, 0, True)
    for mc in range(MC):
        emit_out_half(mc, 1, True)
        if mc < 12:
            store_mc(mc)
        else:
            store_mc(mc, nc.gpsimd)


_CACHE = {}


def build():
    if "nc" in _CACHE:
        return _CACHE["nc"]
    nc = bacc.Bacc("TRN2", target_bir_lowering=False, debug=False)
    with tile.TileContext(nc) as tc, ExitStack() as ctx:
        _emit(nc, tc, ctx)
    nc.compile()
    _CACHE["nc"] = nc
    return nc


def make_in_maps(data, context, u, s, v, w, bias):
    u_b = np.ascontiguousarray(np.asarray(u, dtype=np.float32)).astype(NP_BF16)
    s = np.ascontiguousarray(np.asarray(s, dtype=np.float32))
    vT_b = np.ascontiguousarray(np.asarray(v, dtype=np.float32).T).astype(NP_BF16)
    w_b = np.ascontiguousarray(np.asarray(w, dtype=np.float32)).astype(NP_BF16)
    bias = np.ascontiguousarray(np.asarray(bias, dtype=np.float32))
    data = np.asarray(data, dtype=np.float32)
    context = np.asarray(context, dtype=np.float32)
    in_maps = []
    for c in range(NCORES):
        sl = slice(c * NB, (c + 1) * NB)
        in_maps.append(
            {
                "dataT": np.ascontiguousarray(data[sl].T).astype(NP_BF16),
                "ctxT": np.ascontiguousarray(context[sl].T).astype(NP_BF16),
                "u": u_b,
                "s": s,
                "vT": vT_b,
                "w": w_b,
                "bias": bias,
            }
        )
    return in_maps


def kernel(data, context, u, s, v, w, bias):
    nc = build()
    in_maps = make_in_maps(data, context, u, s, v, w, bias)
    res = run_bass_kernel_spmd(nc, in_maps, core_ids=list(range(NCORES)))
    return np.concatenate(
        [np.asarray(r["outT"]).astype(np.float32).T for r in res.results], axis=0
    )


# revision 18
# speedup vs baseline: 1.0691x; 1.0691x over previous
"""Trainium2 Bass kernel for nn_CADense (context-adaptive low-rank dense layer).

Computes, for the full batch:
    s_mod = s + context @ w          # [B, R]
    low   = (data @ u) * s_mod       # [B, R]
    out   = relu(low @ v.T + 2*bias) # [B, UNITS]

Sharding: data-parallel over batch across 8 NeuronCores; u/s/v/w/bias
replicated. Each core runs the same Bass program on its 1024-row shard.

The kernel sits at the bf16 ridge: ~31us of PE streaming (73728 512-col
matmul columns at 2.4GHz) vs ~33us of HBM traffic (7.25 MiB loads +
4 MiB stores at ~350-430 GB/s/core). Schedule:

- Loads are chopped into ~0.25 MiB quanta and strictly alternated
  between the sync and scalar HWDGE rings IN CONSUMPTION ORDER: each
  ring's FIFO order is the consumption order (ring order = priority),
  both rings carry equal byte prefixes at every point (no priority
  inversion), push latency halves, and the two queues together can
  exceed the single-queue ~353 GB/s.
- Phase 1 (DMA-bound): smod0 + rank0 stream data as it arrives; the
  ~3us 1.2GHz HAM ramp hides inside the DMA pacing. Garbage warm-up
  matmuls bridge the initial descriptor-push latency, and garbage
  "keeper" matmuls plug the remaining bubbles - any PE idle gap over
  ~2us makes the HAM power manager throttle the PE to 50% for
  microseconds at a time.
- Phase 2 (PE-bound): out halves (relu(vT.T @ lowT + 2b) per (m-chunk,
  batch-half)) interleave with the DMA-paced rank1 stream; after mul1
  the remaining halves complete each m-chunk's full-batch row block,
  which stores with 2KB-contiguous runs (2x the descriptor efficiency
  of half-batch stores), FIFO behind the loads on the two rings; the
  last chunks ride the idle gpsimd ring.
- outT stays transposed so the 2*bias term is a per-partition scalar
  fused into PSUM evacuation (scalar.activation bias / DVE max+add),
  alternating scalar/vector per half so neither engine serializes.
"""

import os
import sys
from contextlib import ExitStack

import numpy as np
import ml_dtypes


def _ensure_concourse():
    try:
        import concourse  # noqa: F401
    except ImportError:
        for p in ("/opt/trn_rl_repo", "/root/.axon_site/_ro/trn_rl_repo"):
            if os.path.isdir(p) and p not in sys.path:
                sys.path.insert(0, p)


_ensure_concourse()

import concourse.tile as tile  # noqa: E402
from concourse import bacc, mybir  # noqa: E402
from concourse.bass_utils import run_bass_kernel_spmd  # noqa: E402

NCORES = 8
B, N_IN, UNITS, RANK, CCTX = 8192, 2048, 2048, 256, 512
NB = B // NCORES  # batch rows per core
P = 128
BT = 512  # batch tile (free dim of rank matmuls, PSUM bank width)
NBT = NB // BT  # 2 batch tiles per core
KC = N_IN // P  # 16 contraction chunks for data @ u
CC = CCTX // P  # 4 contraction chunks for context @ w
RC = RANK // P  # 2 rank chunks
MC = UNITS // P  # 16 output unit chunks (partition dim of outT)
N_WARMUP_MM = 9  # 512-col garbage matmuls bridging the push-latency window

F32 = mybir.dt.float32
BF16 = mybir.dt.bfloat16
NP_BF16 = np.dtype(ml_dtypes.bfloat16)


def _emit(nc, tc, ctx):
    d_dataT = nc.dram_tensor("dataT", [N_IN, NB], BF16, kind="ExternalInput")
    d_ctxT = nc.dram_tensor("ctxT", [CCTX, NB], BF16, kind="ExternalInput")
    d_u = nc.dram_tensor("u", [N_IN, RANK], BF16, kind="ExternalInput")
    d_s = nc.dram_tensor("s", [RANK], F32, kind="ExternalInput")
    d_vT = nc.dram_tensor("vT", [RANK, UNITS], BF16, kind="ExternalInput")
    d_w = nc.dram_tensor("w", [CCTX, RANK], BF16, kind="ExternalInput")
    d_bias = nc.dram_tensor("bias", [UNITS], F32, kind="ExternalInput")
    d_outT = nc.dram_tensor("outT", [UNITS, NB], BF16, kind="ExternalOutput")

    ap_dataT = d_dataT.ap().rearrange("(c p) b -> p c b", p=P)
    ap_ctxT = d_ctxT.ap().rearrange("(cc p) b -> p cc b", p=P)
    ap_u = d_u.ap().rearrange("(q p) r -> p q r", p=P)
    ap_w = d_w.ap().rearrange("(cc p) r -> p cc r", p=P)
    ap_vT = d_vT.ap().rearrange("(rc p) m -> p rc m", p=P)
    ap_outT = d_outT.ap().rearrange("(mc p) b -> p mc b", p=P)

    singles = ctx.enter_context(tc.tile_pool(name="singles", bufs=1))
    du_psum = ctx.enter_context(tc.tile_pool(name="du_psum", bufs=2, space="PSUM"))
    s_psum = ctx.enter_context(tc.tile_pool(name="s_psum", bufs=2, space="PSUM"))
    o_psum = ctx.enter_context(tc.tile_pool(name="o_psum", bufs=4, space="PSUM"))

    # Warm-up operands, memset on the (otherwise idle) gpsimd engine so
    # the first LDWEIGHTS can issue right after the entry barrier.
    wu_a = singles.tile([P, P], BF16, name="wu_a")
    wu_b = singles.tile([P, BT], BF16, name="wu_b")
    nc.gpsimd.memset(wu_a[:], 1.0)
    nc.gpsimd.memset(wu_b[:], 1.0)

    # ---- SBUF tiles ----------------------------------------------------
    u_t = {g: singles.tile([P, 4, RANK], BF16, name=f"u{g}") for g in range(4)}
    # All data in 2-kc chunks (0.25 MiB quanta for the 2-ring alternation).
    d_t = {
        (bt, g): singles.tile([P, 2, BT], BF16, name=f"d{bt}g{g}")
        for bt in range(NBT)
        for g in range(KC // 2)
    }

    def data_chunk(bt, kc):
        return d_t[(bt, kc // 2)][:, kc % 2, :]

    s_sb = singles.tile([P, RC], F32, name="s_sb")
    bias_sb = singles.tile([P, MC], F32, name="bias_sb")
    bias2 = singles.tile([P, MC], F32, name="bias2")
    nbias2 = singles.tile([P, MC], F32, name="nbias2")
    w_sb = singles.tile([P, CC, RANK], BF16, name="w_sb")
    ctx_t = {bt: singles.tile([P, CC, BT], BF16, name=f"ctx{bt}") for bt in range(NBT)}
    vT_sb = singles.tile([P, RC, UNITS], BF16, name="vT_sb")

    smod = singles.tile([P, RC, NB], F32, name="smod")
    lowT = {
        (bt, rc): singles.tile([P, BT], BF16, name=f"lowT{bt}r{rc}")
        for bt in range(NBT)
        for rc in range(RC)
    }
    # Full-batch output staging: one row block per m-chunk, so the store
    # for m-chunk mc is a single [P, NB] transfer with 2KB-contiguous
    # destination rows.
    osb = {mc: singles.tile([P, NB], BF16, name=f"osb{mc}") for mc in range(MC)}

    # ---- load queue: ONE hardware ring (sync), consumption order --------
    # A single HWDGE queue saturates the per-core HBM read bandwidth;
    # splitting loads across queues round-robins the DMA engines between
    # streams and measurably LOWERS total throughput while destroying the
    # ring-order priority. Also: never push loads from the scalar engine -
    # it is the evacuation engine, and pushes (which block on ring-full /
    # sem-recycle waits) would serialize ahead of the evacuations.
    def ld(out, in_):
        nc.sync.dma_start(out=out, in_=in_)

    def load_data2(bt, g):
        ld(d_t[(bt, g)][:], ap_dataT[:, 2 * g : 2 * g + 2, bt * BT : (bt + 1) * BT])

    ld(ctx_t[0][:, 0:2, :], ap_ctxT[:, 0:2, 0:BT])
    ld(w_sb[:], ap_w)
    ld(u_t[0][:], ap_u[:, 0:4])
    load_data2(0, 0)  # kc0-1
    ld(ctx_t[0][:, 2:4, :], ap_ctxT[:, 2:4, 0:BT])
    load_data2(0, 1)  # kc2-3
    ld(s_sb[:], d_s.ap().rearrange("(rc p) -> p rc", p=P))
    ld(u_t[1][:], ap_u[:, 4:8])
    load_data2(0, 2)  # kc4-5
    ld(bias_sb[:], d_bias.ap().rearrange("(mc p) -> p mc", p=P))
    load_data2(0, 3)  # kc6-7
    ld(u_t[2][:], ap_u[:, 8:12])
    load_data2(0, 4)  # kc8-9
    load_data2(0, 5)  # kc10-11
    ld(u_t[3][:], ap_u[:, 12:16])
    load_data2(0, 6)  # kc12-13
    load_data2(0, 7)  # kc14-15
    ld(ctx_t[1][:], ap_ctxT[:, :, BT:])
    ld(vT_sb[:, :, 0:1024], ap_vT[:, :, 0:1024])  # mc0-7, both rc
    ld(vT_sb[:, :, 1024:2048], ap_vT[:, :, 1024:2048])  # mc8-15
    for g in range(8):
        load_data2(1, g)

    # ---- compute stages ------------------------------------------------
    wu_ps = o_psum.tile([P, BT], F32, tag="po", name="wu_ps")

    def emit_warmups(n):
        for _ in range(n):
            nc.tensor.matmul(wu_ps[:], lhsT=wu_a[:], rhs=wu_b[:], start=True, stop=True)

    def emit_keepers(n):
        """Small garbage matmuls that keep the PE busy across DMA-paced
        bubbles so the HAM power manager never sees a long idle gap."""
        for _ in range(n):
            nc.tensor.matmul(
                wu_ps[:, 0:P], lhsT=wu_a[:], rhs=wu_b[:, 0:P], start=True, stop=True
            )

    pd_t = {}
    ps_t = {}

    def emit_smod_mms(bt, cc_lo, cc_hi):
        """ctx @ w matmuls, cc-outer so they chase the ctx halves."""
        if cc_lo == 0:
            ps_t[bt] = [s_psum.tile([P, BT], F32, tag="ps", name="ps") for _ in range(RC)]
        for cc in range(cc_lo, cc_hi):
            for rc in range(RC):
                nc.tensor.matmul(
                    ps_t[bt][rc][:],
                    lhsT=w_sb[:, cc, rc * P : (rc + 1) * P],
                    rhs=ctx_t[bt][:, cc, :],
                    start=(cc == 0),
                    stop=(cc == CC - 1),
                )

    def emit_smod_evac(bt):
        for rc in range(RC):
            nc.scalar.add(
                smod[:, rc, bt * BT : (bt + 1) * BT],
                ps_t[bt][rc][:],
                add=s_sb[:, rc : rc + 1],
            )

    def emit_rank_mms(bt, kc_lo, kc_hi, keepers=0):
        if kc_lo == 0:
            pd_t[bt] = [du_psum.tile([P, BT], F32, tag="pd", name="pd") for _ in range(RC)]
        for kc in range(kc_lo, kc_hi):
            for rc in range(RC):
                nc.tensor.matmul(
                    pd_t[bt][rc][:],
                    lhsT=u_t[kc // 4][:, kc % 4, rc * P : (rc + 1) * P],
                    rhs=data_chunk(bt, kc),
                    start=(kc == 0),
                    stop=(kc == KC - 1),
                )
            if keepers and kc % 2 == 1:
                emit_keepers(keepers)

    def emit_mul(bt):
        """lowT = pd * smod on the vector engine (bf16 out)."""
        for rc in range(RC):
            nc.vector.tensor_mul(
                out=lowT[(bt, rc)][:],
                in0=pd_t[bt][rc][:],
                in1=smod[:, rc, bt * BT : (bt + 1) * BT],
            )

    # Output halves: outT[mc, bt-half] = relu(vT.T @ lowT + 2*bias) into a
    # rotating PSUM bank, evacuated into the full-batch osb[mc] row block.
    _opools = [(o_psum, "po"), (s_psum, "ps"), (o_psum, "po"), (du_psum, "pd")]
    _oidx = [0]

    def emit_out_half(mc, bt, borrow_all):
        if borrow_all:
            pool, tag = _opools[_oidx[0] % 4]
        else:
            pool, tag = o_psum, "po"
        _oidx[0] += 1
        po = pool.tile([P, BT], F32, tag=tag, name="po")
        for rc in range(RC):
            nc.tensor.matmul(
                po[:],
                lhsT=vT_sb[:, rc, mc * P : (mc + 1) * P],
                rhs=lowT[(bt, rc)][:],
                start=(rc == 0),
                stop=(rc == RC - 1),
            )
        dst = osb[mc][:, bt * BT : (bt + 1) * BT]
        if (mc + bt) % 2 == 0:
            nc.scalar.activation(
                dst,
                po[:],
                mybir.ActivationFunctionType.Relu,
                bias=bias2[:, mc : mc + 1],
            )
        else:
            nc.vector.tensor_scalar(
                out=dst,
                in0=po[:],
                scalar1=nbias2[:, mc : mc + 1],
                scalar2=bias2[:, mc : mc + 1],
                op0=mybir.AluOpType.max,
                op1=mybir.AluOpType.add,
            )

    def store_mc(mc, eng=None):
        eng = eng or nc.sync
        eng.dma_start(out=ap_outT[:, mc, :], in_=osb[mc][:])

    # ---- software pipeline, PE emission in DMA-arrival order -----------
    emit_warmups(N_WARMUP_MM)
    emit_smod_mms(0, 0, 2)  # ctx0 first half + w
    emit_rank_mms(0, 0, 2, keepers=1)  # u0, d0 kc0-1
    emit_smod_mms(0, 2, 4)  # ctx0 second half
    emit_smod_evac(0)
    nc.scalar.mul(bias2[:], bias_sb[:], 2.0)
    nc.scalar.mul(nbias2[:], bias_sb[:], -2.0)
    emit_rank_mms(0, 2, 16, keepers=1)  # d0 kc2-15
    emit_mul(0)
    emit_keepers(8)
    emit_smod_mms(1, 0, 4)  # ctx1
    emit_smod_evac(1)
    emit_keepers(8)
    # Phase 2a: bt0 output halves (need only mul0 + vT) interleaved with
    # the DMA-paced rank1 stream.
    emit_out_half(0, 0, False)
    emit_out_half(1, 0, False)
    for g in range(8):
        emit_rank_mms(1, 2 * g, 2 * g + 2)
        if g < 8:
            emit_out_half(2 + g, 0, False)
    emit_mul(1)
    # Phase 2b: remaining bt0 halves, then bt1 halves completing each
    # m-chunk row block, stored immediately with 2KB-contiguous runs.
    for mc in range(10, MC):
        emit_out_half(mc, 0, True)
    for mc in range(MC):
        emit_out_half(mc, 1, True)
        if mc < 12:
            store_mc(mc)
        elif mc < 15:
            store_mc(mc, nc.gpsimd)
        else:
            store_mc(mc, nc.scalar)


_CACHE = {}


def build():
    if "nc" in _CACHE:
        return _CACHE["nc"]
    nc = bacc.Bacc("TRN2", target_bir_lowering=False, debug=False)
    with tile.TileContext(nc) as tc, ExitStack() as ctx:
        _emit(nc, tc, ctx)
    nc.compile()
    _CACHE["nc"] = nc
    return nc


def make_in_maps(data, context, u, s, v, w, bias):
    u_b = np.ascontiguousarray(np.asarray(u, dtype=np.float32)).astype(NP_BF16)
    s = np.ascontiguousarray(np.asarray(s, dtype=np.float32))
    vT_b = np.ascontiguousarray(np.asarray(v, dtype=np.float32).T).astype(NP_BF16)
    w_b = np.ascontiguousarray(np.asarray(w, dtype=np.float32)).astype(NP_BF16)
    bias = np.ascontiguousarray(np.asarray(bias, dtype=np.float32))
    data = np.asarray(data, dtype=np.float32)
    context = np.asarray(context, dtype=np.float32)
    in_maps = []
    for c in range(NCORES):
        sl = slice(c * NB, (c + 1) * NB)
        in_maps.append(
            {
                "dataT": np.ascontiguousarray(data[sl].T).astype(NP_BF16),
                "ctxT": np.ascontiguousarray(context[sl].T).astype(NP_BF16),
                "u": u_b,
                "s": s,
                "vT": vT_b,
                "w": w_b,
                "bias": bias,
            }
        )
    return in_maps


def kernel(data, context, u, s, v, w, bias):
    nc = build()
    in_maps = make_in_maps(data, context, u, s, v, w, bias)
    res = run_bass_kernel_spmd(nc, in_maps, core_ids=list(range(NCORES)))
    return np.concatenate(
        [np.asarray(r["outT"]).astype(np.float32).T for r in res.results], axis=0
    )
